# revision 16
# baseline (speedup 1.0000x reference)
"""Trainium2 Bass kernel: batched 1-D linear interpolation on a uniform grid.

out[b, j] = y[b, i_j] + w_j * (y[b, i_j + 1] - y[b, i_j])

i_j / w_j depend only on x_new, so the host folds them into a sparse
selection-matrix S [NUM_POINTS, M] with exactly two nonzeros per column
((1-w_j) at row i_j, w_j at row i_j+1) and the device computes the gather+lerp
as one dense matmul  out = y @ S  on the TensorEngine (bf16 in, fp32 PSUM out).
This replaces the GPSIMD ap_gather path (which was the 3.7 ms bottleneck) with
~0.44 ms of PE work per core.

The host ships y pre-transposed/cast to bf16 in b-tile-major blocks so batch
tile b only depends on its own 512 KB slice of yT (the 16 MB S stream is the
only long pole at kernel start): for batch tile b and 128-row grid chunk k,
lhsT = yT[b][:, k, :] (stationary) and rhs = S[k][:, bank] (moving),
accumulating over k into 8 PSUM banks (8 x 512 = M columns).

Sharding: pure data parallel over the batch axis across 8 NeuronCores
(y_points rows 16384 -> 8 x 2048); x_new-derived constants are replicated.
"""

import numpy as np

BATCH = 16384
NUM_POINTS = 2048
M = 4096
N_CORES = 8
ROWS_PER_CORE = BATCH // N_CORES  # 2048
P = 128
N_BTILES = ROWS_PER_CORE // P  # 16 batch tiles per core
N_KCHUNKS = NUM_POINTS // P  # 16 contraction chunks
N_BANKS = 8  # PSUM banks; 8 x 512 fp32 = M
BANK = M // N_BANKS  # 512

_NC_CACHE = {}


def _build_nc():
    import concourse.bacc as bacc
    import concourse.mybir as mybir
    from concourse.tile import TileContext

    f32 = mybir.dt.float32
    bf16 = mybir.dt.bfloat16

    nc = bacc.Bacc()
    # yT[bt, p, k, b] = y[128*bt + b, 128*k + p] as bf16 (host transpose+cast);
    # flattened to [P, N_BTILES * N_KCHUNKS * P] partition-major per b-block.
    yT = nc.dram_tensor(
        "yT", [P, N_BTILES * N_KCHUNKS * P], bf16, kind="ExternalInput"
    )
    # s[p, k, j] = S[128*k + p, j] as bf16
    s = nc.dram_tensor("s", [P, N_KCHUNKS * M], bf16, kind="ExternalInput")
    out = nc.dram_tensor("out", [ROWS_PER_CORE, M], f32, kind="ExternalOutput")

    with TileContext(nc) as tc:
        with (
            tc.tile_pool(name="const", bufs=1) as cp,
            tc.tile_pool(name="psum", bufs=1, space="PSUM") as pp,
            tc.tile_pool(name="outp", bufs=4) as op,
        ):
            yT_t = cp.tile([P, N_BTILES, N_KCHUNKS, P], bf16, tag="yT")
            s_t = cp.tile([P, N_KCHUNKS, M], bf16, tag="s")
            # yT blocks 0+1 and then the S chunk stream first (everything is
            # FIFO on the sync HWDGE ring): b-tile 0 only waits on ~2 MB and
            # b-tile 1 is ready before b-tile 0's matmuls finish; remaining yT
            # blocks stream behind the S chunks.
            nc.sync.dma_start(
                out=yT_t[:, 0],
                in_=yT[:, : N_KCHUNKS * P].rearrange("p (k b) -> p k b", k=N_KCHUNKS),
            )
            # chunk 0 split per half so the very first matmuls start sooner
            nc.sync.dma_start(out=s_t[:, 0, : M // 2], in_=s[:, : M // 2])
            nc.sync.dma_start(out=s_t[:, 0, M // 2 :], in_=s[:, M // 2 : M])
            nc.sync.dma_start(
                out=yT_t[:, 1],
                in_=yT[
                    :, N_KCHUNKS * P : 2 * N_KCHUNKS * P
                ].rearrange("p (k b) -> p k b", k=N_KCHUNKS),
            )
            for k in range(1, N_KCHUNKS):
                nc.sync.dma_start(out=s_t[:, k, :], in_=s[:, k * M : (k + 1) * M])
            for b in range(2, N_BTILES):
                nc.sync.dma_start(
                    out=yT_t[:, b],
                    in_=yT[
                        :, b * N_KCHUNKS * P : (b + 1) * N_KCHUNKS * P
                    ].rearrange("p (k b) -> p k b", k=N_KCHUNKS),
                )

            def store_bank(b, i, psum):
                o_t = op.tile([P, BANK], f32, tag="o", name="o_t")
                nc.any.tensor_copy(out=o_t[:], in_=psum[:])
                # scalar ring: keeps stores off the input-laden sync ring
                nc.scalar.dma_start(
                    out=out[b * P : (b + 1) * P, i * BANK : (i + 1) * BANK],
                    in_=o_t[:],
                )

            # b-tile 0 runs while S streams in, so all 8 banks advance
            # together as chunks land (k-outer over the full M).
            psums = [
                pp.tile([P, BANK], f32, tag=f"ps{i}", name=f"ps{i}")
                for i in range(N_BANKS)
            ]
            for k in range(N_KCHUNKS):
                lhsT = yT_t[:, 0, k, :]
                for i in range(N_BANKS):
                    nc.tensor.matmul(
                        psums[i],
                        lhsT,
                        s_t[:, k, i * BANK : (i + 1) * BANK],
                        start=(k == 0),
                        stop=(k == N_KCHUNKS - 1),
                    )
            for i in range(N_BANKS):
                store_bank(0, i, psums[i])

            # b-tiles 1..15 pipeline as half-tiles: banks 0-3 hold columns
            # [0, 2048), banks 4-7 hold [2048, 4096).  A half's copies drain a
            # full half-tile (~14 us) before its banks are reused, so the PE
            # never waits on PSUM.
            for b in range(1, N_BTILES):
                for h in range(2):
                    psums = [
                        pp.tile([P, BANK], f32, tag=f"ps{4 * h + i}", name=f"ps{4 * h + i}")
                        for i in range(4)
                    ]
                    for k in range(N_KCHUNKS):
                        lhsT = yT_t[:, b, k, :]
                        for i in range(4):
                            nc.tensor.matmul(
                                psums[i],
                                lhsT,
                                s_t[:, k, (4 * h + i) * BANK : (4 * h + i + 1) * BANK],
                                start=(k == 0),
                                stop=(k == N_KCHUNKS - 1),
                            )
                    for i in range(4):
                        store_bank(b, 4 * h + i, psums[i])

    nc.compile()
    return nc


def _get_nc():
    if "nc" not in _NC_CACHE:
        _NC_CACHE["nc"] = _build_nc()
    return _NC_CACHE["nc"]


def _host_precompute(x_new):
    """Replicate the reference's searchsorted/weight math with the same jax
    ops on the same backend, so boundary decisions and weight rounding match
    the reference bit-for-bit (the device searchsorted/divide are not IEEE-
    exact, so numpy does NOT reproduce them)."""
    import jax.numpy as jnp

    x_new_j = jnp.asarray(np.asarray(x_new, dtype=np.float32))
    x_points = jnp.linspace(0.0, 1.0, NUM_POINTS, dtype=x_new_j.dtype)
    idxs = jnp.searchsorted(x_points, x_new_j, side="right") - 1
    idxs = jnp.clip(idxs, 0, NUM_POINTS - 2)
    x1 = x_points[idxs]
    x2 = x_points[idxs + 1]
    w = (x_new_j - x1) / (x2 - x1)
    return np.asarray(idxs).astype(np.int64), np.asarray(w, dtype=np.float32)


def _make_in_maps(y_points, x_new):
    import ml_dtypes

    bf16 = ml_dtypes.bfloat16
    idxs, w = _host_precompute(np.asarray(x_new))

    # Selection matrix S [NUM_POINTS, M]: two nonzeros per column.
    S = np.zeros((NUM_POINTS, M), dtype=np.float32)
    cols = np.arange(M)
    S[idxs, cols] = 1.0 - w
    S[idxs + 1, cols] = w
    # partition-major layout [P, k, j]
    s_pl = np.ascontiguousarray(
        S.reshape(N_KCHUNKS, P, M).transpose(1, 0, 2).reshape(P, N_KCHUNKS * M)
    ).astype(bf16)

    y_full = np.asarray(y_points, dtype=np.float32)
    in_maps = []
    for c in range(N_CORES):
        y_c = y_full[c * ROWS_PER_CORE : (c + 1) * ROWS_PER_CORE]  # [b, grid]
        # yT_pl[p, bt, k, b] = y_c[128*bt + b, 128*k + p]
        yT_pl = np.ascontiguousarray(
            y_c.reshape(N_BTILES, P, N_KCHUNKS, P).transpose(3, 0, 2, 1)
            .reshape(P, N_BTILES * N_KCHUNKS * P)
        ).astype(bf16)
        in_maps.append({"yT": yT_pl, "s": s_pl})
    return in_maps


def run(y_points, x_new, trace=False, **spmd_kwargs):
    """Run the Bass kernel; returns (output, BassKernelResults)."""
    from concourse.bass_utils import run_bass_kernel_spmd

    nc = _get_nc()
    in_maps = _make_in_maps(y_points, x_new)
    res = run_bass_kernel_spmd(
        nc, in_maps, list(range(N_CORES)), trace=trace, **spmd_kwargs
    )
    out = np.concatenate([r["out"] for r in res.results], axis=0)
    return out, res


def kernel(y_points, x_new):
    out, _ = run(y_points, x_new)
    return out


# revision 18
# speedup vs baseline: 1.2098x; 1.2098x over previous
"""Trainium2 Bass kernel: batched 1-D linear interpolation on a uniform grid.

out[b, j] = y[b, i_j] + w_j * (y[b, i_j + 1] - y[b, i_j])

i_j / w_j depend only on x_new, so the host folds them into a sparse
selection-matrix S [NUM_POINTS, M] with exactly two nonzeros per column
((1-w_j) at row i_j, w_j at row i_j+1) and the device computes the gather+lerp
as one dense matmul  out = y @ S  on the TensorEngine (bf16 in, fp32 PSUM out).
This replaces the GPSIMD ap_gather path (which was the 3.7 ms bottleneck) with
~0.44 ms of PE work per core.

The host ships y pre-transposed/cast to bf16 in b-tile-major blocks so batch
tile b only depends on its own 512 KB slice of yT (the 16 MB S stream is the
only long pole at kernel start): for batch tile b and 128-row grid chunk k,
lhsT = yT[b][:, k, :] (stationary) and rhs = S[k][:, bank] (moving),
accumulating over k into 8 PSUM banks (8 x 512 = M columns).

Sharding: pure data parallel over the batch axis across 8 NeuronCores
(y_points rows 16384 -> 8 x 2048); x_new-derived constants are replicated.
"""

import numpy as np

BATCH = 16384
NUM_POINTS = 2048
M = 4096
N_CORES = 8
ROWS_PER_CORE = BATCH // N_CORES  # 2048
P = 128
N_BTILES = ROWS_PER_CORE // P  # 16 batch tiles per core
N_KCHUNKS = NUM_POINTS // P  # 16 contraction chunks
N_BANKS = 8  # PSUM banks; 8 x 512 fp32 = M
BANK = M // N_BANKS  # 512

_NC_CACHE = {}


def _build_nc():
    import concourse.bacc as bacc
    import concourse.mybir as mybir
    from concourse.tile import TileContext

    f32 = mybir.dt.float32
    bf16 = mybir.dt.bfloat16

    nc = bacc.Bacc()
    # yT[bt, p, k, b] = y[128*bt + b, 128*k + p] as bf16 (host transpose+cast);
    # flattened to [P, N_BTILES * N_KCHUNKS * P] partition-major per b-block.
    yT = nc.dram_tensor(
        "yT", [P, N_BTILES * N_KCHUNKS * P], bf16, kind="ExternalInput"
    )
    # s[p, k, j] = S[128*k + p, j] as bf16
    s = nc.dram_tensor("s", [P, N_KCHUNKS * M], bf16, kind="ExternalInput")
    out = nc.dram_tensor("out", [ROWS_PER_CORE, M], f32, kind="ExternalOutput")

    with TileContext(nc) as tc:
        with (
            tc.tile_pool(name="const", bufs=1) as cp,
            tc.tile_pool(name="psum", bufs=1, space="PSUM") as pp,
            tc.tile_pool(name="outp", bufs=4) as op,
        ):
            yT_t = cp.tile([P, N_BTILES, N_KCHUNKS, P], bf16, tag="yT")
            s_t = cp.tile([P, N_KCHUNKS, M], bf16, tag="s")
            # yT blocks 0+1 and then the S chunk stream first (everything is
            # FIFO on the sync HWDGE ring): b-tile 0 only waits on ~2 MB and
            # b-tile 1 is ready before b-tile 0's matmuls finish; remaining yT
            # blocks stream behind the S chunks.
            for b in range(2):
                nc.sync.dma_start(
                    out=yT_t[:, b],
                    in_=yT[
                        :, b * N_KCHUNKS * P : (b + 1) * N_KCHUNKS * P
                    ].rearrange("p (k b) -> p k b", k=N_KCHUNKS),
                )
            # S streams as half-chunks, columns [0, M/2) of every chunk first:
            # during the load phase b-tiles 0+1 each consume a half-chunk with
            # 4 matmuls, so each arriving 0.5 MB enables 8 matmuls (~1.7 us of
            # PE per ~1.2 us of DMA) and the PE never starves.
            for h in range(2):
                for k in range(N_KCHUNKS):
                    nc.sync.dma_start(
                        out=s_t[:, k, h * (M // 2) : (h + 1) * (M // 2)],
                        in_=s[:, k * M + h * (M // 2) : k * M + (h + 1) * (M // 2)],
                    )
            for b in range(2, N_BTILES):
                nc.sync.dma_start(
                    out=yT_t[:, b],
                    in_=yT[
                        :, b * N_KCHUNKS * P : (b + 1) * N_KCHUNKS * P
                    ].rearrange("p (k b) -> p k b", k=N_KCHUNKS),
                )

            def store_bank(b, i, psum):
                o_t = op.tile([P, BANK], f32, tag="o", name="o_t")
                nc.any.tensor_copy(out=o_t[:], in_=psum[:])
                # scalar ring: keeps stores off the input-laden sync ring
                nc.scalar.dma_start(
                    out=out[b * P : (b + 1) * P, i * BANK : (i + 1) * BANK],
                    in_=o_t[:],
                )

            # load phase: b-tiles 0+1 run concurrently at half width (b0 on
            # banks 0-3, b1 on banks 4-7) against the same arriving S bytes.
            for h in range(2):
                ps0 = [
                    pp.tile([P, BANK], f32, tag=f"ps{i}", name=f"ps{i}")
                    for i in range(4)
                ]
                ps1 = [
                    pp.tile([P, BANK], f32, tag=f"ps{4 + i}", name=f"ps{4 + i}")
                    for i in range(4)
                ]
                for k in range(N_KCHUNKS):
                    for psums, b in ((ps0, 0), (ps1, 1)):
                        lhsT = yT_t[:, b, k, :]
                        for i in range(4):
                            nc.tensor.matmul(
                                psums[i],
                                lhsT,
                                s_t[:, k, (4 * h + i) * BANK : (4 * h + i + 1) * BANK],
                                start=(k == 0),
                                stop=(k == N_KCHUNKS - 1),
                            )
                for i in range(4):
                    store_bank(0, 4 * h + i, ps0[i])
                for i in range(4):
                    store_bank(1, 4 * h + i, ps1[i])

            # b-tiles 2..15 pipeline as half-tiles: banks 0-3 hold columns
            # [0, 2048), banks 4-7 hold [2048, 4096).  A half's copies drain a
            # full half-tile (~14 us) before its banks are reused, so the PE
            # never waits on PSUM.
            for b in range(2, N_BTILES):
                for h in range(2):
                    psums = [
                        pp.tile([P, BANK], f32, tag=f"ps{4 * h + i}", name=f"ps{4 * h + i}")
                        for i in range(4)
                    ]
                    for k in range(N_KCHUNKS):
                        lhsT = yT_t[:, b, k, :]
                        for i in range(4):
                            nc.tensor.matmul(
                                psums[i],
                                lhsT,
                                s_t[:, k, (4 * h + i) * BANK : (4 * h + i + 1) * BANK],
                                start=(k == 0),
                                stop=(k == N_KCHUNKS - 1),
                            )
                    for i in range(4):
                        store_bank(b, 4 * h + i, psums[i])

    nc.compile()
    return nc


def _get_nc():
    if "nc" not in _NC_CACHE:
        _NC_CACHE["nc"] = _build_nc()
    return _NC_CACHE["nc"]


def _host_precompute(x_new):
    """Replicate the reference's searchsorted/weight math with the same jax
    ops on the same backend, so boundary decisions and weight rounding match
    the reference bit-for-bit (the device searchsorted/divide are not IEEE-
    exact, so numpy does NOT reproduce them)."""
    import jax.numpy as jnp

    x_new_j = jnp.asarray(np.asarray(x_new, dtype=np.float32))
    x_points = jnp.linspace(0.0, 1.0, NUM_POINTS, dtype=x_new_j.dtype)
    idxs = jnp.searchsorted(x_points, x_new_j, side="right") - 1
    idxs = jnp.clip(idxs, 0, NUM_POINTS - 2)
    x1 = x_points[idxs]
    x2 = x_points[idxs + 1]
    w = (x_new_j - x1) / (x2 - x1)
    return np.asarray(idxs).astype(np.int64), np.asarray(w, dtype=np.float32)


def _make_in_maps(y_points, x_new):
    import ml_dtypes

    bf16 = ml_dtypes.bfloat16
    idxs, w = _host_precompute(np.asarray(x_new))

    # Selection matrix S [NUM_POINTS, M]: two nonzeros per column.
    S = np.zeros((NUM_POINTS, M), dtype=np.float32)
    cols = np.arange(M)
    S[idxs, cols] = 1.0 - w
    S[idxs + 1, cols] = w
    # partition-major layout [P, k, j]
    s_pl = np.ascontiguousarray(
        S.reshape(N_KCHUNKS, P, M).transpose(1, 0, 2).reshape(P, N_KCHUNKS * M)
    ).astype(bf16)

    y_full = np.asarray(y_points, dtype=np.float32)
    in_maps = []
    for c in range(N_CORES):
        y_c = y_full[c * ROWS_PER_CORE : (c + 1) * ROWS_PER_CORE]  # [b, grid]
        # yT_pl[p, bt, k, b] = y_c[128*bt + b, 128*k + p]
        yT_pl = np.ascontiguousarray(
            y_c.reshape(N_BTILES, P, N_KCHUNKS, P).transpose(3, 0, 2, 1)
            .reshape(P, N_BTILES * N_KCHUNKS * P)
        ).astype(bf16)
        in_maps.append({"yT": yT_pl, "s": s_pl})
    return in_maps


def run(y_points, x_new, trace=False, **spmd_kwargs):
    """Run the Bass kernel; returns (output, BassKernelResults)."""
    from concourse.bass_utils import run_bass_kernel_spmd

    nc = _get_nc()
    in_maps = _make_in_maps(y_points, x_new)
    res = run_bass_kernel_spmd(
        nc, in_maps, list(range(N_CORES)), trace=trace, **spmd_kwargs
    )
    out = np.concatenate([r["out"] for r in res.results], axis=0)
    return out, res


def kernel(y_points, x_new):
    out, _ = run(y_points, x_new)
    return out
